# revision 2
# baseline (speedup 1.0000x reference)
"""2-layer GAT (graph attention) on 8 Trainium2 NeuronCores — v2.

Sharding (head x row-half), as baseline: core c owns head c%4 and query rows
[(c//4)*2048 : +2048). Changes vs baseline:

- Phase B (h = x@W1) runs fp8e4 DoubleRow (PE 4x): xT/w1e shipped fp8, W
  pre-scaled x8 on host, un-scaled at the PSUM->SBUF copy. h stored fp8
  (values only; 6% elem quantization averages out over 2048-key attention).
- Attention scores, S-path blocks: the mask rides pre-exp as an additive
  -240*(1-m) (host-encoded), so exp(leakyrelu(z-240(1-m))-4) is exactly the
  masked numerator: DVE does one add, ScalarE does Prelu+Exp straight to
  fp8, no post-mask multiply. The -4 shift keeps exp() inside fp8e4 range.
- V-path blocks keep the rank-1 identity on DVE (tensor_scalar 4x +
  scalar_tensor_tensor + mask multiply), pm in bf16; matmuls are mixed-dtype
  (bf16 stationary x fp8 moving h).
- S-path pm is fp8 in pair tiles -> DoubleRow attention matmuls (PE 4x).
- Layer 2 is restructured in transposed form: stat h2g [128,17] x moving
  pm2 [128,512] -> psum [17,512], 32 matmuls instead of 128; log-softmax in
  the transposed layout with a PE ones-colsum (no max subtraction needed:
  logits are bounded), final 16x128 transposes.
- Phase E half-1 (relu/transpose/proj/payload) is emitted between RS2-issue
  and RS2-consume so it fills the RS2 latency; the payload AllGather is
  split in two so AG1 hides under E2 and layer-2 elementwise on AG1's keys
  hides AG2.
"""

import os

import numpy as np
import ml_dtypes

_STOP = os.environ.get("K_STOP", "")       # bench-only: B, D0, D, E
_DO_RS = os.environ.get("K_RS", "1") == "1"
NSP = int(os.environ.get("K_NSP", "8"))    # S-pairs per half (2*NSP S-blocks)
NS2 = int(os.environ.get("K_NS2", "16"))   # S-blocks in layer 2 (of 32)
B8 = os.environ.get("K_B8", "0") == "1"    # fp8 DoubleRow phase B
SF8 = os.environ.get("K_SF8", "1") == "1"  # S-path pm in fp8 (DoubleRow)
AGS = os.environ.get("K_AGS", "1") == "1"  # split AllGather
NOMASK = os.environ.get("K_NOMASK", "0") == "1"  # diag: no per-block mask DMA
PM0 = os.environ.get("K_PM0", "0") == "1"  # diag: static pm tiles, PE floor
VF8 = os.environ.get("K_VF8", "0") == "1"  # V-blocks paired fp8 DoubleRow

import concourse.bass as bass
import concourse.tile as tile
from concourse import bacc, mybir
from concourse.bass_utils import run_bass_kernel_spmd
from concourse.masks import make_identity

P = 128
N, F, O, H, C = 4096, 512, 256, 4, 16
NCORES = 8
QL = N // 2              # 2048 query rows per core (layer 1)
QH = QL // 2             # 1024-row attention half
Q2 = N // NCORES         # 512 layer-2 rows per core
MB = N // P              # 32 key blocks
KB = F // P              # 4 contraction blocks over F
OB = O // P              # 2 contraction blocks over O
ALPHA = 0.2
PAY = C + 2              # payload cols: 0:16 h2, 16 ones, 17 e2_dst
ESHIFT = 4.0             # exp(lr(z)-ESHIFT): keeps fp8 pm in range
W1SC = 8.0               # host pre-scale on W1 for fp8 (subnormal dodge)
MBIG = 240.0             # additive mask: -MBIG*(1-m) pre-leakyrelu

bf16 = mybir.dt.bfloat16
f32 = mybir.dt.float32
f8 = mybir.dt.float8e4
AF = mybir.ActivationFunctionType
ALU = mybir.AluOpType
AX = mybir.AxisListType
DR = mybir.MatmulPerfMode.DoubleRow

RS_GROUPS = [[0, 1, 2, 3], [4, 5, 6, 7]]


def _sched(nblocks, npairs):
    """Interleaved item list: ('S', kb) pairs (kb, kb+1) and ('V', kb)
    singles — or ('W', kb) V-pairs when VF8."""
    items = []
    if VF8:
        nw = nblocks // 2 - npairs
        order = []
        acc = 0.0
        per = nw / npairs if npairs else 0.0
        for _ in range(npairs):
            order.append("S")
            acc += per
            while acc >= 1.0 and len(order) < nblocks // 2:
                order.append("W")
                acc -= 1.0
        while len(order) < nblocks // 2:
            order.append("W")
        items = [(t, 2 * i) for i, t in enumerate(order)]
        return items
    nv = nblocks - 2 * npairs
    kb = 0
    acc = 0.0
    per = nv / npairs if npairs else 0.0
    for _ in range(npairs):
        items.append(("S", kb))
        kb += 2
        acc += per
        while acc >= 1.0 and kb < nblocks:
            items.append(("V", kb))
            kb += 1
            acc -= 1.0
    while kb < nblocks:
        items.append(("V", kb))
        kb += 1
    assert sorted(k for t, k in items for k in ((k, k + 1) if t == "S" else (k,))) \
        == list(range(nblocks))
    return items


def _sched2(nblocks, ns):
    """Layer-2 schedule: singles, ns S-blocks spread among nblocks."""
    if ns <= 0:
        return [("V", kb) for kb in range(nblocks)]
    stride = nblocks / ns
    sset = {min(nblocks - 1, int(i * stride)) for i in range(ns)}
    return [("S" if kb in sset else "V", kb) for kb in range(nblocks)]


ITEMS1 = _sched(MB, NSP)
S_SET1 = {k for t, kb in ITEMS1 if t == "S" for k in (kb, kb + 1)}
ITEMS2 = _sched2(MB, NS2)
S_SET2 = {kb for t, kb in ITEMS2 if t == "S"}

XDT = f8 if B8 else bf16


def _build(reps=1):
    nc = bacc.Bacc("TRN2", target_bir_lowering=False, debug=False,
                   num_devices=NCORES)

    xT_d = nc.dram_tensor("xT", [F, N], XDT, kind="ExternalInput").ap()
    xTq_d = nc.dram_tensor("xTq", [F, QL], bf16, kind="ExternalInput").ap()
    w1e_d = nc.dram_tensor("w1e", [F, O + 1], XDT, kind="ExternalInput").ap()
    wsrc_d = nc.dram_tensor("wsrc", [F, 1], bf16, kind="ExternalInput").ap()
    maskT_d = nc.dram_tensor("maskT", [N, QL], bf16, kind="ExternalInput").ap()
    mask2T_d = nc.dram_tensor("mask2T", [N, Q2], bf16,
                              kind="ExternalInput").ap()
    w2p_d = nc.dram_tensor("w2p", [O, PAY], bf16, kind="ExternalInput").ap()
    out_d = nc.dram_tensor("out", [Q2, C], f32, kind="ExternalOutput").ap()

    with tile.TileContext(nc) as tc:
        for _ in range(reps):
            _emit(tc, xT_d, xTq_d, w1e_d, wsrc_d, maskT_d, mask2T_d, w2p_d,
                  out_d)
    nc.compile()
    return nc


def _emit(tc, xT_d, xTq_d, w1e_d, wsrc_d, maskT_d, mask2T_d, w2p_d, out_d):
    nc = tc.nc
    hsc = 1.0 / W1SC if B8 else 1.0
    with tc.tile_pool(name="singles", bufs=1) as singles:
        # ---- persistent SBUF tensors ----
        xT_sb = singles.tile([P, KB, N], XDT)
        xTq_sb = singles.tile([P, KB, QL], bf16)
        w1e_sb = singles.tile([P, KB, O + 1], XDT)
        wsrc_sb = singles.tile([P, KB, 1], bf16)
        w2p_sb = singles.tile([P, OB, PAY], bf16)
        ones1 = singles.tile([1, P], bf16)
        ones16 = singles.tile([16, 1], f32)
        col16 = singles.tile([1, 16], f32)
        negsh = singles.tile([P, 1], f32)
        ident = singles.tile([P, P], bf16)
        identf = singles.tile([C, C], f32)
        h8 = singles.tile([P, MB, O + 1], f8)    # h | ones col, fp8
        edst = singles.tile([P, MB], f32)
        Ek = singles.tile([P, MB], f32)
        Fk = singles.tile([P, MB], f32)
        esbb = singles.tile([P, QL], bf16)       # broadcast es (unshifted)
        Eqb = singles.tile([P, QL], bf16)        # exp(es - ESHIFT)
        Fqb = singles.tile([P, QL], bf16)        # exp(a*es - ESHIFT)
        esrow = singles.tile([1, QL], bf16)
        x2p = singles.tile([P, 16, O], bf16)
        x2raw = singles.tile([P, 4, O], bf16)
        x2_sb = singles.tile([P, 4, O], bf16)
        x2T = singles.tile([P, OB, Q2], bf16)
        pay_sb = singles.tile([P, 4, PAY], bf16)
        h2g_sb = singles.tile([P, MB, PAY], bf16)
        e2d_all = singles.tile([P, MB], f32)
        Ek2 = singles.tile([P, MB], f32)
        Fk2 = singles.tile([P, MB], f32)
        esbb2 = singles.tile([P, Q2], bf16)
        Eq2b = singles.tile([P, Q2], bf16)
        Fq2b = singles.tile([P, Q2], bf16)
        e2row = singles.tile([1, Q2], bf16)
        mask2_sb = singles.tile([P, MB, Q2], bf16)

        # ---- input DMAs: critical-path tensors first ----
        nc.sync.dma_start(wsrc_sb[:],
                          wsrc_d.rearrange("(kb p) c -> p kb c", p=P))
        nc.sync.dma_start(xTq_sb[:], xTq_d.rearrange("(kb p) q -> p kb q", p=P))
        nc.sync.dma_start(w1e_sb[:], w1e_d.rearrange("(kb p) c -> p kb c", p=P))
        nc.sync.dma_start(w2p_sb[:], w2p_d.rearrange("(ob p) c -> p ob c", p=P))
        xT_r = xT_d.rearrange("(kb p) n -> p kb n", p=P)
        for g in range(8):
            s = bass.ts(g, N // 8)
            nc.sync.dma_start(xT_sb[:, :, s], xT_r[:, :, s])
        maskT_r = maskT_d.rearrange("(b p) q -> p b q", p=P)
        mask2T_r = mask2T_d.rearrange("(b p) q -> p b q", p=P)

        nc.vector.memset(ones1[:], 1.0)
        nc.vector.memset(ones16[:], 1.0)
        nc.vector.memset(col16[:], 1.0)
        nc.vector.memset(negsh[:], -ESHIFT)
        make_identity(nc, ident[:])
        make_identity(nc, identf[:])
        nc.vector.memset(h8[:, :, O], float(H))

        # ---- phase C: es row, broadcast, Eq/Fq (concurrent with B) ----
        with tc.tile_pool(name="es_psum", bufs=1, space="PSUM") as epp, \
             tc.tile_pool(name="bc_psum", bufs=1, space="PSUM") as bpp:
            for ch in range(4):
                pse = epp.tile([1, 512], f32, tag="ps_es")
                for kb in range(KB):
                    nc.tensor.matmul(pse[:], wsrc_sb[:, kb, :],
                                     xTq_sb[:, kb, bass.ts(ch, 512)],
                                     start=(kb == 0), stop=(kb == KB - 1))
                nc.vector.tensor_copy(esrow[:, bass.ts(ch, 512)], pse[:])
            psB = bpp.tile([P, QL], f32, tag="psB")
            for ch in range(4):
                nc.tensor.matmul(psB[:, bass.ts(ch, 512)], ones1[:],
                                 esrow[:, bass.ts(ch, 512)],
                                 start=True, stop=True)
            nc.vector.tensor_copy(esbb[:], psB[:])
            nc.scalar.activation(Eqb[:], psB[:], AF.Exp, bias=negsh[:])
            nc.scalar.activation(Fqb[:], psB[:], AF.Exp, scale=ALPHA,
                                 bias=negsh[:])

            # ---- phase B: h and e_dst for ALL key rows (local) ----
            with tc.tile_pool(name="h_psum", bufs=3, space="PSUM") as hpp:
                for nb in range(MB):
                    ps = hpp.tile([P, O + 1], f32, tag="ps_h")
                    if B8:
                        for j in range(KB // 2):
                            nc.tensor.matmul(
                                ps[:], xT_sb[:, 2 * j:2 * j + 2,
                                             bass.ts(nb, P)],
                                w1e_sb[:, 2 * j:2 * j + 2, :],
                                start=(j == 0), stop=(j == KB // 2 - 1),
                                perf_mode=DR)
                    else:
                        for kb in range(KB):
                            nc.tensor.matmul(ps[:],
                                             xT_sb[:, kb, bass.ts(nb, P)],
                                             w1e_sb[:, kb, :],
                                             start=(kb == 0),
                                             stop=(kb == KB - 1))
                    if nb % 2 == 0:
                        nc.vector.tensor_scalar_mul(h8[:, nb, 0:O],
                                                    ps[:, 0:O], hsc)
                    else:
                        nc.scalar.activation(h8[:, nb, 0:O], ps[:, 0:O],
                                             AF.Copy, scale=hsc)
                    nc.vector.tensor_scalar_mul(edst[:, nb:nb + 1],
                                                ps[:, O:O + 1], hsc)
                for ch in range(4):
                    s = bass.ts(ch, MB // 4)
                    nc.scalar.activation(Ek[:, s], edst[:, s], AF.Exp)
                    nc.scalar.activation(Fk[:, s], edst[:, s], AF.Exp,
                                         scale=ALPHA)

        if _STOP == "B":
            return
        # ---- phase D: layer-1 attention, two query halves + RS each ----
        x2r_ds = []
        with tc.tile_pool(name="dram1", bufs=1, space="DRAM") as dram1:
          with tc.tile_pool(name="acc_psum", bufs=1, space="PSUM") as accp, \
               tc.tile_pool(name="mA_pool", bufs=3) as mAp, \
               tc.tile_pool(name="mV_pool", bufs=6) as mVp, \
               tc.tile_pool(name="pm8_pool", bufs=3) as pm8p, \
               tc.tile_pool(name="pmb_pool", bufs=4) as pmbp, \
               tc.tile_pool(name="zt_pool", bufs=4) as ztp, \
               tc.tile_pool(name="small1", bufs=4) as sp1:
            for qh in range(2):
                qs = bass.ts(qh, QH)
                accs = [accp.tile([P, O + 1], f32, tag=f"acc{qc}",
                                  name=f"acc{qc}") for qc in range(8)]
                if PM0:
                    pm8s = pm8p.tile([P, 2, QH], f8 if SF8 else bf16,
                                     tag="pm8", name="pm8")
                    nc.vector.memset(pm8s[:], 0.25)
                    pmbs = pmbp.tile([P, QH], bf16, tag="pm", name="pm")
                    nc.vector.memset(pmbs[:], 0.25)
                if NOMASK:
                    mtA0 = mAp.tile([P, 2, QH], bf16, tag="mtA", name="mtA")
                    nc.sync.dma_start(mtA0[:, 0, :], maskT_r[:, 0, qs])
                    nc.sync.dma_start(mtA0[:, 1, :], maskT_r[:, 1, qs])
                    mt0 = mVp.tile([P, QH], bf16, tag="mt", name="mt")
                    nc.sync.dma_start(mt0[:], maskT_r[:, 2, qs])
                nit = len(ITEMS1)
                for idx, (kind, kb) in enumerate(ITEMS1):
                    first, last = idx == 0, idx == nit - 1
                    if kind == "S" and PM0:
                        pm8 = pm8s
                    elif kind == "V" and PM0:
                        pm = pmbs
                    if kind == "S" and not PM0:
                        if NOMASK:
                            mtA = mtA0
                        else:
                            mtA = mAp.tile([P, 2, QH], bf16, tag="mtA",
                                           name="mtA")
                            nc.sync.dma_start(mtA[:, 0, :],
                                              maskT_r[:, kb, qs])
                            nc.sync.dma_start(mtA[:, 1, :],
                                              maskT_r[:, kb + 1, qs])
                        if SF8:
                            pm8 = pm8p.tile([P, 2, QH], f8, tag="pm8",
                                            name="pm8")
                        else:
                            pm8 = pm8p.tile([P, 2, QH], bf16, tag="pm8",
                                            name="pm8")
                        for i in range(2):
                            zm = ztp.tile([P, QH], bf16, tag="zm", name="zm")
                            nc.vector.tensor_add(zm[:], esbb[:, qs],
                                                 mtA[:, i, :])
                            zz = ztp.tile([P, QH], bf16, tag="zz", name="zz")
                            nc.scalar.activation(
                                zz[:], zm[:], AF.Prelu,
                                bias=edst[:, kb + i:kb + i + 1],
                                scale=1.0, alpha=ALPHA)
                            nc.scalar.activation(pm8[:, i, :], zz[:], AF.Exp,
                                                 bias=negsh[:])
                    elif kind == "W" and not PM0:
                        mtA = mAp.tile([P, 2, QH], bf16, tag="mtA",
                                       name="mtA")
                        nc.sync.dma_start(mtA[:, 0, :], maskT_r[:, kb, qs])
                        nc.sync.dma_start(mtA[:, 1, :], maskT_r[:, kb + 1, qs])
                        pm8 = pm8p.tile([P, 2, QH], f8, tag="pm8",
                                        name="pm8")
                        for i in range(2):
                            t1 = ztp.tile([P, QH], bf16, tag="t1", name="t1")
                            nc.vector.tensor_scalar_mul(t1[:], Eqb[:, qs],
                                                        Ek[:, kb + i:kb + i + 1])
                            t1f = ztp.tile([P, QH], bf16, tag="t1f",
                                           name="t1f")
                            nc.vector.tensor_scalar_mul(t1f[:], Fqb[:, qs],
                                                        Fk[:, kb + i:kb + i + 1])
                            t2 = ztp.tile([P, QH], bf16, tag="t2", name="t2")
                            nc.vector.tensor_max(t2[:], t1[:], t1f[:])
                            nc.vector.tensor_mul(pm8[:, i, :], t2[:],
                                                 mtA[:, i, :])
                    elif kind == "V" and not PM0:
                        if NOMASK:
                            mt = mt0
                        else:
                            mt = mVp.tile([P, QH], bf16, tag="mt", name="mt")
                            nc.sync.dma_start(mt[:], maskT_r[:, kb, qs])
                        t1 = ztp.tile([P, QH], bf16, tag="t1", name="t1")
                        nc.vector.tensor_scalar_mul(t1[:], Eqb[:, qs],
                                                    Ek[:, kb:kb + 1])
                        t1f = ztp.tile([P, QH], bf16, tag="t1f", name="t1f")
                        nc.vector.tensor_scalar_mul(t1f[:], Fqb[:, qs],
                                                    Fk[:, kb:kb + 1])
                        t2 = ztp.tile([P, QH], bf16, tag="t2", name="t2")
                        nc.vector.tensor_max(t2[:], t1[:], t1f[:])
                        pm = pmbp.tile([P, QH], bf16, tag="pm", name="pm")
                        nc.vector.tensor_mul(pm[:], t2[:], mt[:])
                    if kind in ("S", "W"):
                        if SF8 or kind == "W":
                            for qc in range(8):
                                nc.tensor.matmul(
                                    accs[qc][:], pm8[:, :, bass.ts(qc, P)],
                                    h8[:, kb:kb + 2, :],
                                    start=first, stop=last, perf_mode=DR)
                        else:
                            for i in range(2):
                                for qc in range(8):
                                    nc.tensor.matmul(
                                        accs[qc][:],
                                        pm8[:, i, bass.ts(qc, P)],
                                        h8[:, kb + i, :],
                                        start=first, stop=last)
                    else:
                        for qc in range(8):
                            nc.tensor.matmul(accs[qc][:],
                                             pm[:, bass.ts(qc, P)],
                                             h8[:, kb, :],
                                             start=first, stop=last)
                for qc in range(8):
                    r = sp1.tile([P, 1], f32, tag="r")
                    nc.vector.reciprocal(r[:], accs[qc][:, O:O + 1])
                    nc.vector.tensor_scalar_mul(x2p[:, qh * 8 + qc, :],
                                                accs[qc][:, 0:O], r[:])
                x2h_d = dram1.tile([QH, O], bf16, name=f"x2h{qh}")
                x2r_d = dram1.tile([QH // 4, O], bf16, name=f"x2r{qh}")
                nc.sync.dma_start(
                    x2h_d.rearrange("(b p) c -> p b c", p=P),
                    x2p[:, qh * 8:(qh + 1) * 8, :])
                if _DO_RS:
                    nc.gpsimd.collective_compute(
                        "ReduceScatter", ALU.add, replica_groups=RS_GROUPS,
                        ins=[x2h_d.opt()], outs=[x2r_d.opt()])
                x2r_ds.append(x2r_d)
                if _STOP == "D0" and qh == 0:
                    return
          if _STOP == "D":
              return
          # mask2 loads now (after phase-D mask queue pressure)
          for g in range(4):
              s = bass.ts(g, MB // 4)
              nc.sync.dma_start(mask2_sb[:, s, :], mask2T_r[:, s, :])

          # ---- phase E (split halves to fill RS latency) + AllGather ----
          with tc.tile_pool(name="dram2", bufs=1, space="DRAM") as dram2:
           with tc.tile_pool(name="l2_psum", bufs=2, space="PSUM") as lpp:
            pay_d = dram2.tile([Q2, PAY], bf16)
            if AGS:
                gath_ds = [dram2.tile([N // 2, PAY], bf16,
                                      addr_space="Shared", name=f"g{i}")
                           for i in range(2)]
            else:
                gath_ds = [dram2.tile([N, PAY], bf16, addr_space="Shared",
                                      name="gath")]
            pay_r = pay_d.rearrange("(b p) c -> p b c", p=P)

            for eh in range(2):
                qcs = (0, 1) if eh == 0 else (2, 3)
                nc.sync.dma_start(
                    x2raw[:, eh * 2:(eh + 1) * 2, :],
                    x2r_ds[eh].rearrange("(b p) c -> p b c", p=P))
                nc.vector.tensor_relu(x2_sb[:, eh * 2:(eh + 1) * 2, :],
                                      x2raw[:, eh * 2:(eh + 1) * 2, :])
                for qc in qcs:
                    for ob in range(OB):
                        tp = lpp.tile([P, P], bf16, tag="tp")
                        nc.tensor.transpose(tp[:],
                                            x2_sb[:, qc, bass.ts(ob, P)],
                                            ident[:])
                        nc.vector.tensor_copy(x2T[:, ob, bass.ts(qc, P)],
                                              tp[:])
                for qc in qcs:
                    ps2 = lpp.tile([P, C + 1], f32, tag="ps2")
                    for ob in range(OB):
                        nc.tensor.matmul(ps2[:], x2T[:, ob, bass.ts(qc, P)],
                                         w2p_sb[:, ob, 0:C + 1],
                                         start=(ob == 0), stop=(ob == OB - 1))
                    nc.vector.tensor_copy(pay_sb[:, qc, 0:C], ps2[:, 0:C])
                    nc.vector.tensor_copy(pay_sb[:, qc, C + 1:C + 2],
                                          ps2[:, C:C + 1])
                    nc.vector.memset(pay_sb[:, qc, C:C + 1], 1.0)
                if AGS:
                    s = bass.ts(eh, 2)
                    nc.sync.dma_start(pay_r[:, s, :],
                                      pay_sb[:, eh * 2:(eh + 1) * 2, :])
                    nc.gpsimd.collective_compute(
                        "AllGather", ALU.bypass,
                        replica_groups=[list(range(NCORES))],
                        ins=[pay_d[bass.ts(eh, Q2 // 2), :].opt()],
                        outs=[gath_ds[eh].opt()])
            if not AGS:
                nc.sync.dma_start(pay_r[:], pay_sb[:])
                nc.gpsimd.collective_compute(
                    "AllGather", ALU.bypass,
                    replica_groups=[list(range(NCORES))],
                    ins=[pay_d.opt()], outs=[gath_ds[0].opt()])

            # local query-side layer-2 terms (need only RS outputs)
            ps_e2 = lpp.tile([1, Q2], f32, tag="ps_e2")
            for ob in range(OB):
                nc.tensor.matmul(ps_e2[:], w2p_sb[:, ob, C + 1:C + 2],
                                 x2T[:, ob, :],
                                 start=(ob == 0), stop=(ob == OB - 1))
            nc.vector.tensor_copy(e2row[:], ps_e2[:])
            psB2 = lpp.tile([P, Q2], f32, tag="psB2")
            nc.tensor.matmul(psB2[:], ones1[:], e2row[:],
                             start=True, stop=True)
            nc.vector.tensor_copy(esbb2[:], psB2[:])
            nc.scalar.activation(Eq2b[:], psB2[:], AF.Exp)
            nc.scalar.activation(Fq2b[:], psB2[:], AF.Exp, scale=ALPHA)

            if _STOP == "E":
                return
            # ---- phase F: layer-2 attention (transposed) ----
            for gh, g_d in enumerate(gath_ds):
                nblk = MB // len(gath_ds)
                nc.sync.dma_start(
                    h2g_sb[:, gh * nblk:(gh + 1) * nblk, :],
                    g_d.rearrange("(b p) c -> p b c", p=P))
                s = bass.ts(gh, nblk) if len(gath_ds) == 2 else bass.ts(0, MB)
                nc.vector.tensor_copy(e2d_all[:, s],
                                      h2g_sb[:, s, C + 1])
                nc.scalar.activation(Ek2[:, s], e2d_all[:, s], AF.Exp)
                nc.scalar.activation(Fk2[:, s], e2d_all[:, s], AF.Exp,
                                     scale=ALPHA)

            with tc.tile_pool(name="f_psum", bufs=1, space="PSUM") as fpp, \
                 tc.tile_pool(name="z2_pool", bufs=6) as z2p, \
                 tc.tile_pool(name="small2", bufs=1) as sp2:
                out2 = fpp.tile([C + 1, Q2], f32, tag="out2")
                for idx, (kind, kb) in enumerate(ITEMS2):
                    first, last = idx == 0, idx == len(ITEMS2) - 1
                    m2 = mask2_sb[:, kb, :]
                    pm2 = z2p.tile([P, Q2], bf16, tag="pm2", name="pm2")
                    if kind == "S":
                        zm2 = z2p.tile([P, Q2], bf16, tag="zm2", name="zm2")
                        nc.vector.tensor_add(zm2[:], esbb2[:], m2)
                        z2 = z2p.tile([P, Q2], bf16, tag="z2", name="z2")
                        nc.scalar.activation(z2[:], zm2[:], AF.Prelu,
                                             bias=e2d_all[:, kb:kb + 1],
                                             scale=1.0, alpha=ALPHA)
                        nc.scalar.activation(pm2[:], z2[:], AF.Exp)
                    else:
                        t1 = z2p.tile([P, Q2], bf16, tag="t12", name="t12")
                        nc.vector.tensor_scalar_mul(t1[:], Eq2b[:],
                                                    Ek2[:, kb:kb + 1])
                        t1f = z2p.tile([P, Q2], bf16, tag="t1f2", name="t1f2")
                        nc.vector.tensor_scalar_mul(t1f[:], Fq2b[:],
                                                    Fk2[:, kb:kb + 1])
                        t2 = z2p.tile([P, Q2], bf16, tag="t22", name="t22")
                        nc.vector.tensor_max(t2[:], t1[:], t1f[:])
                        nc.vector.tensor_mul(pm2[:], t2[:], m2)
                    nc.tensor.matmul(out2[:], h2g_sb[:, kb, 0:C + 1], pm2[:],
                                     start=first, stop=last)

                # transposed log-softmax (no max subtraction: logits bounded)
                o2s = sp2.tile([C + 1, Q2], f32, tag="o2s")
                nc.vector.tensor_copy(o2s[:], out2[:])
                r2f = sp2.tile([1, Q2], f32, tag="r2f")
                nc.vector.reciprocal(r2f[:], o2s[C:C + 1, :])
                r2b = fpp.tile([C, Q2], f32, tag="r2b")
                nc.tensor.matmul(r2b[:], col16[:], r2f[:], start=True,
                                 stop=True)
                logitsT = sp2.tile([C, Q2], f32, tag="logitsT")
                nc.vector.tensor_mul(logitsT[:], o2s[0:C, :], r2b[:])
                expP = sp2.tile([C, Q2], f32, tag="expP")
                nc.scalar.activation(expP[:], logitsT[:], AF.Exp)
                ps_s1 = fpp.tile([1, Q2], f32, tag="ps_s1")
                nc.tensor.matmul(ps_s1[:], ones16[:], expP[:], start=True,
                                 stop=True)
                lseL = sp2.tile([1, Q2], f32, tag="lseL")
                nc.scalar.activation(lseL[:], ps_s1[:], AF.Ln)
                ps_l16 = fpp.tile([C, Q2], f32, tag="ps_l16")
                nc.tensor.matmul(ps_l16[:], col16[:], lseL[:], start=True,
                                 stop=True)
                res = sp2.tile([C, Q2], f32, tag="res")
                nc.vector.tensor_sub(res[:], logitsT[:], ps_l16[:])
                res4 = sp2.tile([P, 4, C], f32, tag="res4")
                for b in range(4):
                    pt = fpp.tile([P, C], f32, tag="pt")
                    nc.tensor.transpose(pt[:], res[:, bass.ts(b, P)],
                                        identf[:])
                    nc.vector.tensor_copy(res4[:, b, :], pt[:])
                nc.sync.dma_start(out_d.rearrange("(b p) c -> p b c", p=P),
                                  res4[:])


def out_rows_for_core(c):
    """Global output row indices handled by core c, in on-device order."""
    qb = (c // 4) * QL
    r = c % 4
    rows = list(range(qb + 256 * r, qb + 256 * r + 256))
    rows += list(range(qb + QH + 256 * r, qb + QH + 256 * r + 256))
    return rows


def prep_in_maps(x, adj, W1, a1, W2, a2):
    bf = ml_dtypes.bfloat16
    xdt = ml_dtypes.float8_e4m3 if B8 else bf
    x = np.asarray(x, dtype=np.float32)
    adj = np.asarray(adj)
    W1 = np.asarray(W1, dtype=np.float32)
    a1 = np.asarray(a1, dtype=np.float32)
    W2 = np.asarray(W2, dtype=np.float32)
    a2 = np.asarray(a2, dtype=np.float32)

    xT = np.ascontiguousarray(x.T)                                # [F, N]
    wsrc_all = np.einsum("hfo,ho->fh", W1, a1[:, :O])             # [F, H]
    wdst_all = np.einsum("hfo,ho->fh", W1, a1[:, O:])             # [F, H]
    w2p = np.zeros((O, PAY), np.float32)
    w2p[:, 0:C] = W2[0]
    w2p[:, C] = W2[0] @ a2[0, C:]      # e2_dst vector
    w2p[:, C + 1] = W2[0] @ a2[0, :C]  # e2_src vector
    w2p = w2p.astype(bf)
    adj_on = (adj > 0).astype(np.float32)

    rows_all = [out_rows_for_core(c) for c in range(NCORES)]
    if AGS:
        perm = [r for rows in rows_all for r in rows[0:256]]
        perm += [r for rows in rows_all for r in rows[256:512]]
    else:
        perm = [r for rows in rows_all for r in rows]

    wsc = W1SC if B8 else 1.0
    in_maps = []
    for c in range(NCORES):
        hd, qb = c % 4, (c // 4) * QL
        w1e = np.concatenate([W1[hd], wdst_all[:, hd:hd + 1]], 1) * wsc
        # masks with per-key-block encoding
        m1 = np.ascontiguousarray(adj_on[:, qb:qb + QL])          # [N, QL]
        m1enc = m1.copy()
        for kb in S_SET1:
            sl = slice(kb * P, (kb + 1) * P)
            m1enc[sl] = -MBIG * (1.0 - m1[sl])
        m2 = np.ascontiguousarray(adj_on[np.ix_(perm, rows_all[c])])
        m2enc = m2.copy()
        for kb in S_SET2:
            sl = slice(kb * P, (kb + 1) * P)
            m2enc[sl] = -MBIG * (1.0 - m2[sl])
        in_maps.append({
            "xT": xT.astype(xdt),
            "xTq": np.ascontiguousarray(xT[:, qb:qb + QL]).astype(bf),
            "w1e": w1e.astype(xdt),
            "wsrc": np.ascontiguousarray(wsrc_all[:, hd:hd + 1]).astype(bf),
            "maskT": m1enc.astype(bf),
            "mask2T": m2enc.astype(bf),
            "w2p": w2p,
        })
    return in_maps


def assemble_out(results):
    out = np.empty((N, C), np.float32)
    for c in range(NCORES):
        out[out_rows_for_core(c)] = results[c]["out"]
    return out


_CACHED = None


def _get_nc():
    global _CACHED
    if _CACHED is None:
        _CACHED = _build()
    return _CACHED


def kernel(x, adj, W1, a1, W2, a2):
    in_maps = prep_in_maps(x, adj, W1, a1, W2, a2)
    nc = _get_nc()
    res = run_bass_kernel_spmd(nc, in_maps, core_ids=list(range(NCORES)))
    return assemble_out(res.results)
